# revision 34
# baseline (speedup 1.0000x reference)
"""Trainium2 Bass kernel for a multi-head cross-attention module.

Math (validated vs reference; fp8 path 6.4e-3 in numpy sim):
  Q = x@Wq+bq, K = x@Wk          (N=2048, 8 heads, head_dim=64)
  scores[q,k,h] = <Q[q,h,:], K[k,h,:]>/8
    - spatial bias sb(q): per-query shift along k -> softmax no-op, dropped
    - K bias bk: <Q[q,h],bk[h]> is per-(q,h) shift along k -> softmax
      no-op, dropped (exact)
  A = softmax_k(scores); out[q] = sum_{k,h} A[q,k,h]*U[k,h]/Z[q,h] + bo
  where U[k,h] = mg[k]*(x[k]@Wv_tilde[:,h]+bv_tilde[h]) folds the V
  projection, motion gate and output projection (host-prepped: the
  gate MLP + U are O(N*small), 0.4% of total FLOPs; all O(N*d^2)
  projections and the O(N^2*H) attention run on device).

Sharding: queries split 256/core across 8 cores; K/U replicated.

Per-core dataflow (d = head-pair 0..3 pipelined):
  Q/K projections in fp8e4m3 with DoubleRow perf mode (x, Wq, Wk
  quantized on host; 2 k-subtiles of 128 = 256-deep contraction at
  0.5 cycles/col -> 4x fewer PE column-cycles than bf16) ->
  KT staging (ACT/DVE split) ->
  scores S^T[k,q] per key-tile in bf16, head pair concurrent on PE
  row-groups (64-row contraction at base partitions 0/64) ->
  exp split 3 ways: ACT Exp(scale=1/8) tiles; DVE+GPSIMD "Schraudolph"
  (one tensor_scalar producing the bf16 BITS of exp via int16 convert +
  bitcast; end-to-end rel err contribution ~1e-3) ->
  Z/W matmul against [1|U] with 4x PE column-tiling: key-tile kt goes to
  partition strip 32*(kt%4); batch-0 matmuls use a 32-wide stationary
  with start=True so they zero their whole strip (no separate zeroing
  matmul; garbage-free zw_sb for the fold) -> strips folded by a
  [128,9] 4-stacked-identity matmul (E) which also transposes for the
  final combine.

Walrus 1-wait constraint handled by _legalize_waits; steady-state the
schedule needs <=1 wait per instruction (vector clocks elide repeats).
"""

import numpy as np
import ml_dtypes
from contextlib import ExitStack

import concourse.bass as bass
import concourse.mybir as mybir
import concourse.tile as tile
from concourse.bass_utils import run_bass_kernel_spmd

N = 2048
CIN = 256
DOUT = 512
H = 8
HD = 64
NCORES = 8
NQ = N // NCORES        # 256 queries per core
NKT = N // 128          # 16 key tiles
F32 = mybir.dt.float32
BF16 = mybir.dt.bfloat16
I16 = mybir.dt.int16
F8 = mybir.dt.float8e4
DR = mybir.MatmulPerfMode.DoubleRow

# Schraudolph: bf16bits(exp(s/8)) ~= int16((s + B) * A)
SC_EXP = 0.125
A_IMM = SC_EXP * 128.0 / float(np.log(2.0))
B_IMM = 16249.0 / A_IMM                    # (127*128 - 7)/A

# engine split per d-iteration (tunable): exp tiles t=0..7, KT chunks f=0..3
# (GPSIMD/Pool cannot read PSUM on TRN2, so only ACT/DVE can do exp/staging)
EXP_ENG = ("act", "dve", "act", "dve", "act", "dve", "act", "dve")
KT_ON_ACT = (True, False)
FILL_PRE = 4      # PE p-state warmers before Q-proj (input-DMA wait)
FILL_Q = 2        # warmers between Q-proj and the first K-proj
FILL_K = 2        # warmers between K-proj d0 and the first scores

PACKQ8_LAYOUT = [("xq8", 2 * NQ), ("wq8", 2 * DOUT)]
PACKK_LAYOUT = [("uw", 9 * NKT), ("ef", 9)]
PACKQ8W = sum(w for _, w in PACKQ8_LAYOUT)
PACKKW = sum(w for _, w in PACKK_LAYOUT)

_CACHE = {}


def _build_nc(legalize=True):
    nc = bass.Bass()
    # fp8 x^T in two packed tensors (split by key half so K-proj h=0
    # starts after only the first 0.25MB lands)
    d_xp8 = [nc.declare_dram_parameter(f"pack_x{h}", [128, 2048], F8,
                                       isOutput=False) for h in range(2)]
    d_pq8 = nc.declare_dram_parameter("pack_q8", [128, PACKQ8W], F8,
                                      isOutput=False)
    d_pk8 = nc.declare_dram_parameter("pack_k8", [128, 2 * DOUT], F8,
                                      isOutput=False)
    d_pk = nc.declare_dram_parameter("pack_k", [128, PACKKW], BF16,
                                     isOutput=False)
    d_pf = nc.declare_dram_parameter("pack_f32", [128, 5], F32, isOutput=False)
    d_out = nc.declare_dram_parameter("out", [NQ, 1], F32, isOutput=True)

    with tile.TileContext(nc) as tc:
        with ExitStack() as ctx:
            _body(ctx, tc, d_xp8, d_pq8, d_pk8, d_pk, d_pf, d_out)
    if legalize:
        _legalize_waits(nc)
    return nc


def _legalize_waits(nc):
    """walrus accepts a single sync wait per lowered instruction; split any
    extra waits onto injected same-engine NoOps placed just before."""
    cnt = 0
    skip = ("InstEventSemaphore", "InstNoOp", "InstISA")
    for f in nc.m.functions:
        for bb in f.blocks:
            out = []
            for ins in bb.instructions:
                si = getattr(ins, "sync_info", None)
                waits = list(si.on_wait) if (si is not None and si.on_wait) else []
                if len(waits) >= 2 and type(ins).__name__ not in skip:
                    for w in waits[:-1]:
                        nop = mybir.InstEventSemaphore(
                            name=f"wsplit_{cnt}", ins=[], outs=[])
                        cnt += 1
                        nop.engine = ins.engine
                        nop.sync_info = mybir.SyncInfo(on_wait=[w], on_update=[])
                        out.append(nop)
                    ins.sync_info = mybir.SyncInfo(
                        on_wait=[waits[-1]], on_update=list(si.on_update or []))
                out.append(ins)
            bb.instructions[:] = out
    return nc


def _body(ctx, tc, d_xp8, d_pq8, d_pk8, d_pk, d_pf, d_out):
    nc = tc.nc
    AF = mybir.ActivationFunctionType
    OP = mybir.AluOpType

    const_pool = ctx.enter_context(tc.tile_pool(name="const", bufs=1))
    persist = ctx.enter_context(tc.tile_pool(name="persist", bufs=1))

    # ---- input DMAs (ordered by first consumer: Q-proj, then K-proj d0
    # h0 quarter by quarter, then the rest) ----
    pq8_t = const_pool.tile([128, PACKQ8W], F8)
    nc.sync.dma_start(pq8_t[:], d_pq8[:])
    pk8_t = const_pool.tile([128, 2 * DOUT], F8)
    nc.sync.dma_start(pk8_t[:], d_pk8[:])
    xp8 = [const_pool.tile([128, 2048], F8, name=f"xp{h}", tag=f"xp{h}")
           for h in range(2)]
    nc.sync.dma_start(xp8[0][:], d_xp8[0][:])
    pf = const_pool.tile([128, 5], F32)
    nc.sync.dma_start(pf[:], d_pf[:])
    nc.sync.dma_start(xp8[1][:], d_xp8[1][:])
    pk_t = const_pool.tile([128, PACKKW], BF16)
    nc.sync.dma_start(pk_t[:], d_pk[:])

    offq, o = {}, 0
    for nm, w in PACKQ8_LAYOUT:
        offq[nm] = o
        o += w
    offk, o = {}, 0
    for nm, w in PACKK_LAYOUT:
        offk[nm] = o
        o += w
    # DoubleRow views: [128 part, 2 k-subtiles, free]
    xq8 = pq8_t[:, offq["xq8"]:offq["xq8"] + 2 * NQ].rearrange(
        "p (s n) -> p s n", s=2)
    wq8 = pq8_t[:, offq["wq8"]:offq["wq8"] + 2 * DOUT].rearrange(
        "p (s n) -> p s n", s=2)
    wk8 = pk8_t[:, :].rearrange("p (s n) -> p s n", s=2)
    xT8v = [xp8[h][:, :].rearrange("p (s n) -> p s n", s=2) for h in range(2)]
    xT8h = [[xT8v[h][:, :, i * 512:(i + 1) * 512] for i in range(2)]
            for h in range(2)]
    uw = pk_t[:, offk["uw"]:offk["uw"] + 9 * NKT]
    efold = pk_t[:, offk["ef"]:offk["ef"] + 9]
    bq_col = pf[:, 0:4]
    bo_rep = pf[:, 4:5]

    # ACT warm-up: trigger the exp table load early (overlaps input DMA)
    actw = persist.tile([1, 1], F32)
    zrow = persist.tile([1, 128], BF16)
    nc.vector.memset(zrow[:], 0.0)
    frow = persist.tile([1, 512], BF16)
    nc.vector.memset(frow[:], 0.0)
    nc.scalar.activation(actw[:], zrow[0:1, 0:1], AF.Exp, bias=0.0, scale=1.0)

    # ---- persistent SBUF ----
    QT = [persist.tile([128, NQ], BF16, name=f"QT{d}", tag=f"QT{d}")
          for d in range(4)]
    # KT[d][h]: [128, 1024] half h of K^T for head pair d. d=0 is split
    # into two [128,512] tiles per half so the first scores matmuls only
    # depend on the engine-half that staged their keys (d0 staging is on
    # the critical path; tile-granular deps would serialize it).
    KT = [None] + [[persist.tile([128, 1024], BF16, name=f"KT{d}_{h}",
                                 tag=f"KT{d}_{h}")
                    for h in range(2)] for d in range(1, 4)]
    KT0 = [[persist.tile([128, 512], BF16, name=f"KT0_{h}_{q}",
                         tag=f"KT0_{h}_{q}") for q in range(2)]
           for h in range(2)]
    # p[d][t]: exp'd scores, [128 keys, 1024 = 2kt x 2hh x 256q] bf16
    PP = [[persist.tile([128, 1024], BF16, name=f"p{d}_{t}", tag=f"p{d}_{t}")
           for t in range(8)] for d in range(4)]
    zw_sb = persist.tile([128, 4 * 512], BF16)
    res = persist.tile([128, 2], F32)

    stp = ctx.enter_context(tc.tile_pool(name="stp", bufs=3, space="PSUM"))
    zwp = ctx.enter_context(tc.tile_pool(name="zwp", bufs=1, space="PSUM"))
    ztp = ctx.enter_context(tc.tile_pool(name="ztp", bufs=1, space="PSUM"))

    # zt bank: folds use cols 0..143; PE-warming fillers overwrite the whole
    # bank before any fold runs (start=True resets the entries they touch)
    zt = ztp.tile([128, 512], F32, tag="zt")

    def fill(n):
        """p-state warmers: keep the PE streaming during input-DMA / staging
        waits so the clock ramp (full speed needs ~3us of continuous busy)
        completes before the real work arrives. Depends only on local
        memsets; mathematically dead (folds re-reset their entries)."""
        for _ in range(n):
            nc.tensor.matmul(zt[:, :], zrow[:], frow[:],
                             start=True, stop=True, skip_group_check=True)

    fill(FILL_PRE)

    # ---- Q projection: fp8 DoubleRow, all 4 head-pairs in one stp tile ----
    pq = stp.tile([128, 1024], F32, tag="st")
    for d in range(4):
        nc.tensor.matmul(pq[:, d * NQ:(d + 1) * NQ],
                         wq8[:, :, d * 128:(d + 1) * 128],
                         xq8[:], start=True, stop=True, perf_mode=DR)
    fill(FILL_Q)
    for d in range(4):
        if d % 2 == 0:
            nc.scalar.activation(QT[d][:], pq[:, d * NQ:(d + 1) * NQ],
                                 AF.Identity,
                                 bias=bq_col[:, d:d + 1], scale=1.0)
        else:
            nc.vector.tensor_scalar_add(QT[d][:], pq[:, d * NQ:(d + 1) * NQ],
                                        bq_col[:, d:d + 1])

    def kproj_mm(d, h):
        """half h: key chunks i = 0,1 -> one [128,1024] stp tile (fp8 DR)"""
        pk = stp.tile([128, 1024], F32, tag="st")
        for i in range(2):
            nc.tensor.matmul(pk[:, i * 512:(i + 1) * 512],
                             wk8[:, :, d * 128:(d + 1) * 128],
                             xT8h[h][i][:],
                             start=True, stop=True, perf_mode=DR)
        return pk

    def kstage(d, h, pk):
        if d == 0:
            # d0 staging is on the critical path to the first scores --
            # split each half across both engines and into separate tiles
            nc.scalar.activation(KT0[h][0][:], pk[:, 0:512], AF.Copy,
                                 bias=0.0, scale=1.0)
            nc.vector.tensor_copy(KT0[h][1][:], pk[:, 512:1024])
        elif KT_ON_ACT[h]:
            nc.scalar.activation(KT[d][h][:], pk[:], AF.Copy,
                                 bias=0.0, scale=1.0)
        else:
            nc.vector.tensor_copy(KT[d][h][:], pk[:])

    def kproj(d, h):
        kstage(d, h, kproj_mm(d, h))

    # K projection for d=0 (d+1 is projected during d)
    kproj(0, 0)
    kproj(0, 1)
    fill(FILL_K)

    def scores_tile(d, t):
        """two key tiles (kt=2t, 2t+1), both heads -> st [128, 1024].
        Layout [h0kt0|h0kt1|h1kt0|h1kt1]: the concurrently-running
        head-pair matmuls (row groups 0/64) land in different banks --
        concurrent PE writes into one bank are a device fault."""
        st = stp.tile([128, 1024], F32, tag="st")
        for j in range(2):
            kt = 2 * t + j
            h, o = kt // 8, (kt % 8) * 128
            if d == 0:
                src = KT0[h][o // 512]
                o = o % 512
            else:
                src = KT[d][h]
            for hh in range(2):
                nc.tensor.matmul(
                    st[:, hh * 512 + j * NQ:hh * 512 + (j + 1) * NQ],
                    src[hh * HD:(hh + 1) * HD, o:o + 128],
                    QT[d][hh * HD:(hh + 1) * HD, :])
        return st

    def exp_tile(d, t, st):
        p = PP[d][t]
        if d == 3 and t >= 6:
            # tail: halve latency by splitting the last tiles across engines
            nc.scalar.activation(p[:, 0:512], st[:, 0:512], AF.Exp,
                                 bias=0.0, scale=SC_EXP)
            nc.vector.tensor_scalar(p[:, 512:1024].bitcast(I16),
                                    st[:, 512:1024],
                                    B_IMM, A_IMM, op0=OP.add, op1=OP.mult)
            return
        if EXP_ENG[t] == "act":
            nc.scalar.activation(p[:], st[:], AF.Exp, bias=0.0, scale=SC_EXP)
        else:
            nc.vector.tensor_scalar(p[:].bitcast(I16), st[:],
                                    B_IMM, A_IMM, op0=OP.add, op1=OP.mult)

    def zw_batch(d, zw_d, b):
        """key tiles 4b..4b+3 -> 4 col-tiled strip matmuls. Batch 0 uses a
        32-wide stationary with start=True: resets the full 32-row strip
        (rows 9..31 get finite garbage that the E-fold multiplies by 0)."""
        for s in range(4):
            kt = 4 * b + s
            p = PP[d][kt // 2]
            j = kt % 2
            # moving = [h0 block j | h1 block j]: [128, 2, 256] AP
            pv = p[:].rearrange("p (h jq) -> p h jq", h=2)
            pv = pv[:, :, j * NQ:(j + 1) * NQ]
            so = 32 * s
            if b == 0:
                nc.tensor.matmul(
                    zw_d[so:so + 32, :],
                    uw[:, kt * 9:kt * 9 + 32],
                    pv,
                    start=True, stop=False,
                    tile_position=(0, so), skip_group_check=True)
            else:
                nc.tensor.matmul(
                    zw_d[so:so + 9, :],
                    uw[:, kt * 9:kt * 9 + 9],
                    pv,
                    start=False, stop=(kt == NKT - 1),
                    tile_position=(0, so), skip_group_check=True)

    def zw_store(d, zw_d, on_act=True):
        if d == 3:
            # tail: split across engines to halve the last store's latency
            nc.scalar.activation(zw_sb[:, d * 512:d * 512 + 256],
                                 zw_d[:, 0:256], AF.Copy, bias=0.0, scale=1.0)
            nc.vector.tensor_copy(zw_sb[:, d * 512 + 256:(d + 1) * 512],
                                  zw_d[:, 256:512])
        else:
            nc.scalar.activation(zw_sb[:, d * 512:(d + 1) * 512], zw_d[:],
                                 AF.Copy, bias=0.0, scale=1.0)

    def folds(dd):
        for ch in range(4):
            nc.tensor.matmul(
                zt[:, (4 * dd + ch) * 9:(4 * dd + ch) * 9 + 9],
                zw_sb[:, dd * 512 + ch * 128:dd * 512 + (ch + 1) * 128],
                efold[:], skip_group_check=True)

    # ---- main pipeline ----
    # zw batches lag the score/exp stream by half a d-iteration: every
    # batch consumes exp tiles issued >=2us earlier, so the PE never
    # stalls on a freshly-minted exp. Tail does the last two batches.
    zw = {}

    def zw_b(d, b):
        if b == 0:
            zw[d] = zwp.tile([128, 512], F32, tag="zw", name=f"zw{d}")
        zw_batch(d, zw[d], b)

    for d in range(4):
        def tile_(t):
            exp_tile(d, t, scores_tile(d, t))
        tile_(0)
        tile_(1)
        if d:
            zw_b(d - 1, 2)
        tile_(2)
        if d < 3:
            kproj(d + 1, 0)
        tile_(3)
        if d:
            zw_b(d - 1, 3)
            zw_store(d - 1, zw[d - 1], True)
        tile_(4)
        tile_(5)
        zw_b(d, 0)
        if d:
            folds(d - 1)
        if d < 3:
            kproj(d + 1, 1)
        tile_(6)
        tile_(7)
        zw_b(d, 1)
    zw_b(3, 2)
    zw_b(3, 3)
    zw_store(3, zw[3])
    folds(3)

    # ---- final combine: out[q] = sum_h W[q,h]/Z[q,h] + bo ----
    # zt col = 18H + 9qc + r (H = head, qc = query half, r = 0:Z, 1+H:W)
    ld = ctx.enter_context(tc.tile_pool(name="ld", bufs=1))
    zr = ld.tile([128, 16], F32, tag="zr")
    nc.vector.reciprocal(zr[:], zt[:, 0:136:9])           # (H, qc) pairs
    w_ap = zt[:, 1:1 + 19 * 8].rearrange("p (h r) -> p h r", h=8)[:, :, 0:10:9]
    wz = ld.tile([128, 16], F32, tag="wz")                # layout [qc, H]
    nc.vector.tensor_mul(wz[:].rearrange("p (q h) -> p h q", q=2), w_ap,
                         zr[:].rearrange("p (h q) -> p h q", h=8))
    sm = ld.tile([128, 2], F32, tag="sm")
    nc.vector.reduce_sum(sm[:], wz[:].rearrange("p (q h) -> p q h", q=2),
                         axis=mybir.AxisListType.X)
    nc.vector.tensor_scalar_add(res[:], sm[:], bo_rep[:])
    nc.sync.dma_start(d_out.rearrange("(q p) o -> p (q o)", p=128), res[:])


def _host_prep(inputs):
    f32 = np.float32
    bf = ml_dtypes.bfloat16
    f8 = ml_dtypes.float8_e4m3
    x = np.ascontiguousarray(inputs["x"], dtype=f32)
    Wo0 = inputs["Wo"][:, 0].astype(f32)
    wv_t = (inputs["Wv"].astype(f32) * Wo0[None, :]).reshape(CIN, H, HD).sum(-1)
    bv_t = (inputs["bv"].astype(f32) * Wo0).reshape(H, HD).sum(-1)
    # motion gate (host: O(N*small) input prep)
    mf = np.concatenate([inputs["rel_vel"], inputs["rel_angle"]], 1).astype(f32)
    z = np.maximum(mf @ inputs["Wmg1"].astype(f32) + inputs["bmg1"], 0.0)
    z = z @ inputs["Wmg2"].astype(f32) + inputs["bmg2"]
    mg = 1.0 / (1.0 + np.exp(-z))                      # (N, 1)
    U = mg * (x @ wv_t + bv_t)                         # (N, 8) gated
    uw_full = np.concatenate([np.ones((N, 1), f32), U], 1)   # (N, 9)
    uw_pack = uw_full.reshape(NKT, 128, 9).transpose(1, 0, 2).reshape(128, -1)
    E = np.zeros((128, 9), f32)
    for s in range(4):
        E[32 * s:32 * s + 9, :] = np.eye(9, dtype=f32)
    xt = np.ascontiguousarray(x.T)                     # (256, 2048) f32
    wq = inputs["Wq"].astype(f32)
    wk = inputs["Wk"].astype(f32)
    pfv = np.concatenate(
        [inputs["bq"].astype(f32).reshape(4, 128).T,
         np.full((128, 1), inputs["bo"][0], f32)], axis=1)
    common = dict(
        xt_local=xt,
        pack_f32=np.ascontiguousarray(pfv),
    )
    for h in range(2):
        common[f"pack_x{h}"] = np.ascontiguousarray(
            np.concatenate([xt[0:128, h * 1024:(h + 1) * 1024],
                            xt[128:256, h * 1024:(h + 1) * 1024]],
                           axis=1)).astype(f8)
    common["pack_k8"] = np.ascontiguousarray(
        np.concatenate([wk[0:128], wk[128:256]], axis=1)).astype(f8)
    common["pack_k"] = np.ascontiguousarray(
        np.concatenate([uw_pack, E], axis=1)).astype(bf)
    common["_wq8"] = np.concatenate([wq[0:128], wq[128:256]], axis=1)
    return common


def kernel(**inputs):
    if "nc" not in _CACHE:
        _CACHE["nc"] = _build_nc()
    nc = _CACHE["nc"]
    common = _host_prep(inputs)
    xt = common.pop("xt_local")
    wq8 = common.pop("_wq8")
    f8 = ml_dtypes.float8_e4m3
    in_maps = []
    for i in range(NCORES):
        xq = xt[:, i * NQ:(i + 1) * NQ]
        xq8 = np.concatenate([xq[0:128], xq[128:256]], axis=1)
        packed = np.concatenate([xq8, wq8], axis=1)
        in_maps.append(dict(common,
                            pack_q8=np.ascontiguousarray(packed).astype(f8)))
    res = run_bass_kernel_spmd(nc, in_maps, core_ids=list(range(NCORES)),
                               **_CACHE.get("run_kwargs", {}))
    _CACHE["last_results"] = res
    out = np.concatenate([np.asarray(res.results[i]["out"])[:, 0]
                          for i in range(NCORES)])
    return out.astype(np.float32)


# revision 36
# speedup vs baseline: 1.0288x; 1.0288x over previous
"""Trainium2 Bass kernel for a multi-head cross-attention module.

Math (validated vs reference; fp8 path 6.4e-3 in numpy sim):
  Q = x@Wq+bq, K = x@Wk          (N=2048, 8 heads, head_dim=64)
  scores[q,k,h] = <Q[q,h,:], K[k,h,:]>/8
    - spatial bias sb(q): per-query shift along k -> softmax no-op, dropped
    - K bias bk: <Q[q,h],bk[h]> is per-(q,h) shift along k -> softmax
      no-op, dropped (exact)
  A = softmax_k(scores); out[q] = sum_{k,h} A[q,k,h]*U[k,h]/Z[q,h] + bo
  where U[k,h] = mg[k]*(x[k]@Wv_tilde[:,h]+bv_tilde[h]) folds the V
  projection, motion gate and output projection (host-prepped: the
  gate MLP + U are O(N*small), 0.4% of total FLOPs; all O(N*d^2)
  projections and the O(N^2*H) attention run on device).

Sharding: queries split 256/core across 8 cores; K/U replicated.

Per-core dataflow (d = head-pair 0..3 pipelined):
  Q/K projections in fp8e4m3 with DoubleRow perf mode (x, Wq, Wk
  quantized on host; 2 k-subtiles of 128 = 256-deep contraction in one
  instruction -- on TRN2 hw DoubleRow streams 1 cycle per subtile per
  column, so the win is halved instruction/LDWEIGHTS count, not MACs) ->
  KT staging (ACT/DVE split; d0 split across both engines, it gates the
  first scores) ->
  scores S^T[k,q] per key-tile in bf16, head pair concurrent on PE
  row-groups (64-row contraction at base partitions 0/64) ->
  exp: ACT Exp(scale=1/8) / DVE "Schraudolph" alternating per tile (one
  tensor_scalar producing the bf16 BITS of exp via int16 convert +
  bitcast; rel err contribution ~1e-3; GPSIMD cannot read PSUM on TRN2
  so only these two engines can drain scores) ->
  Z/W matmul against [1|U] with 4x PE column-tiling: key-tile kt goes to
  partition strip 32*(kt%4); batch-0 matmuls use a 32-wide stationary
  with start=True so they zero their whole strip (no separate zeroing
  matmul; garbage-free zw_sb for the fold); zw batches lag the
  score/exp stream by half a d-iteration so they never stall on a
  freshly-minted exp -> strips folded by a [128,9] 4-stacked-identity
  matmul (E) which also transposes for the final combine (d3's last exp
  tiles and store are split across both engines to shorten the drain).

PE p-state: full clock needs ~3us of continuous busy, so dead filler
matmuls (zero-stationary into the zt bank) run during the input-DMA
wait; real work then starts at speed.

Walrus 1-wait constraint handled by _legalize_waits; steady-state the
schedule needs <=1 wait per instruction (vector clocks elide repeats).
"""

import numpy as np
import ml_dtypes
from contextlib import ExitStack

import concourse.bass as bass
import concourse.mybir as mybir
import concourse.tile as tile
from concourse.bass_utils import run_bass_kernel_spmd

N = 2048
CIN = 256
DOUT = 512
H = 8
HD = 64
NCORES = 8
NQ = N // NCORES        # 256 queries per core
NKT = N // 128          # 16 key tiles
F32 = mybir.dt.float32
BF16 = mybir.dt.bfloat16
I16 = mybir.dt.int16
F8 = mybir.dt.float8e4
DR = mybir.MatmulPerfMode.DoubleRow

# Schraudolph: bf16bits(exp(s/8)) ~= int16((s + B) * A)
SC_EXP = 0.125
A_IMM = SC_EXP * 128.0 / float(np.log(2.0))
B_IMM = 16249.0 / A_IMM                    # (127*128 - 7)/A

# engine split per d-iteration (tunable): exp tiles t=0..7, KT chunks f=0..3
# (GPSIMD/Pool cannot read PSUM on TRN2, so only ACT/DVE can do exp/staging)
EXP_ENG = ("act", "dve", "act", "dve", "act", "dve", "act", "dve")
KT_ON_ACT = (True, False)
FILL_PRE = 4      # PE p-state warmers before Q-proj (input-DMA wait)
FILL_Q = 2        # warmers between Q-proj and the first K-proj
FILL_K = 2        # warmers between K-proj d0 and the first scores

PACKQ8_LAYOUT = [("xq8", 2 * NQ), ("wq8", 2 * DOUT)]
PACKK_LAYOUT = [("uw", 9 * NKT), ("ef", 9)]
PACKQ8W = sum(w for _, w in PACKQ8_LAYOUT)
PACKKW = sum(w for _, w in PACKK_LAYOUT)

_CACHE = {}


def _build_nc(legalize=True):
    nc = bass.Bass()
    # fp8 x^T in two packed tensors (split by key half so K-proj h=0
    # starts after only the first 0.25MB lands)
    d_xp8 = [nc.declare_dram_parameter(f"pack_x{h}", [128, 2048], F8,
                                       isOutput=False) for h in range(2)]
    d_pq8 = nc.declare_dram_parameter("pack_q8", [128, PACKQ8W], F8,
                                      isOutput=False)
    d_pk8 = nc.declare_dram_parameter("pack_k8", [128, 2 * DOUT], F8,
                                      isOutput=False)
    d_pk = nc.declare_dram_parameter("pack_k", [128, PACKKW], BF16,
                                     isOutput=False)
    d_pf = nc.declare_dram_parameter("pack_f32", [128, 5], F32, isOutput=False)
    d_out = nc.declare_dram_parameter("out", [NQ, 1], F32, isOutput=True)

    with tile.TileContext(nc) as tc:
        with ExitStack() as ctx:
            _body(ctx, tc, d_xp8, d_pq8, d_pk8, d_pk, d_pf, d_out)
    if legalize:
        _legalize_waits(nc)
    return nc


def _legalize_waits(nc):
    """walrus accepts a single sync wait per lowered instruction; split any
    extra waits onto injected same-engine NoOps placed just before."""
    cnt = 0
    skip = ("InstEventSemaphore", "InstNoOp", "InstISA")
    for f in nc.m.functions:
        for bb in f.blocks:
            out = []
            for ins in bb.instructions:
                si = getattr(ins, "sync_info", None)
                waits = list(si.on_wait) if (si is not None and si.on_wait) else []
                if len(waits) >= 2 and type(ins).__name__ not in skip:
                    for w in waits[:-1]:
                        nop = mybir.InstEventSemaphore(
                            name=f"wsplit_{cnt}", ins=[], outs=[])
                        cnt += 1
                        nop.engine = ins.engine
                        nop.sync_info = mybir.SyncInfo(on_wait=[w], on_update=[])
                        out.append(nop)
                    ins.sync_info = mybir.SyncInfo(
                        on_wait=[waits[-1]], on_update=list(si.on_update or []))
                out.append(ins)
            bb.instructions[:] = out
    return nc


def _body(ctx, tc, d_xp8, d_pq8, d_pk8, d_pk, d_pf, d_out):
    nc = tc.nc
    AF = mybir.ActivationFunctionType
    OP = mybir.AluOpType

    const_pool = ctx.enter_context(tc.tile_pool(name="const", bufs=1))
    persist = ctx.enter_context(tc.tile_pool(name="persist", bufs=1))

    # ---- input DMAs (ordered by first consumer: Q-proj, then K-proj d0
    # h0 quarter by quarter, then the rest) ----
    pq8_t = const_pool.tile([128, PACKQ8W], F8)
    nc.sync.dma_start(pq8_t[:], d_pq8[:])
    pk8_t = const_pool.tile([128, 2 * DOUT], F8)
    nc.sync.dma_start(pk8_t[:], d_pk8[:])
    xp8 = [const_pool.tile([128, 2048], F8, name=f"xp{h}", tag=f"xp{h}")
           for h in range(2)]
    nc.sync.dma_start(xp8[0][:], d_xp8[0][:])
    pf = const_pool.tile([128, 5], F32)
    nc.sync.dma_start(pf[:], d_pf[:])
    nc.sync.dma_start(xp8[1][:], d_xp8[1][:])
    pk_t = const_pool.tile([128, PACKKW], BF16)
    nc.sync.dma_start(pk_t[:], d_pk[:])

    offq, o = {}, 0
    for nm, w in PACKQ8_LAYOUT:
        offq[nm] = o
        o += w
    offk, o = {}, 0
    for nm, w in PACKK_LAYOUT:
        offk[nm] = o
        o += w
    # DoubleRow views: [128 part, 2 k-subtiles, free]
    xq8 = pq8_t[:, offq["xq8"]:offq["xq8"] + 2 * NQ].rearrange(
        "p (s n) -> p s n", s=2)
    wq8 = pq8_t[:, offq["wq8"]:offq["wq8"] + 2 * DOUT].rearrange(
        "p (s n) -> p s n", s=2)
    wk8 = pk8_t[:, :].rearrange("p (s n) -> p s n", s=2)
    xT8v = [xp8[h][:, :].rearrange("p (s n) -> p s n", s=2) for h in range(2)]
    xT8h = [[xT8v[h][:, :, i * 512:(i + 1) * 512] for i in range(2)]
            for h in range(2)]
    uw = pk_t[:, offk["uw"]:offk["uw"] + 9 * NKT]
    efold = pk_t[:, offk["ef"]:offk["ef"] + 9]
    bq_col = pf[:, 0:4]
    bo_rep = pf[:, 4:5]

    # ACT warm-up: trigger the exp table load early (overlaps input DMA)
    actw = persist.tile([1, 1], F32)
    zrow = persist.tile([1, 128], BF16)
    nc.vector.memset(zrow[:], 0.0)
    frow = persist.tile([1, 512], BF16)
    nc.vector.memset(frow[:], 0.0)
    nc.scalar.activation(actw[:], zrow[0:1, 0:1], AF.Exp, bias=0.0, scale=1.0)

    # ---- persistent SBUF ----
    QT = [persist.tile([128, NQ], BF16, name=f"QT{d}", tag=f"QT{d}")
          for d in range(4)]
    # KT[d][h]: [128, 1024] half h of K^T for head pair d
    KT = [[persist.tile([128, 1024], BF16, name=f"KT{d}_{h}", tag=f"KT{d}_{h}")
           for h in range(2)] for d in range(4)]
    # p[d][t]: exp'd scores, [128 keys, 1024 = 2kt x 2hh x 256q] bf16
    PP = [[persist.tile([128, 1024], BF16, name=f"p{d}_{t}", tag=f"p{d}_{t}")
           for t in range(8)] for d in range(4)]
    zw_sb = persist.tile([128, 4 * 512], BF16)
    res = persist.tile([128, 2], F32)

    stp = ctx.enter_context(tc.tile_pool(name="stp", bufs=3, space="PSUM"))
    zwp = ctx.enter_context(tc.tile_pool(name="zwp", bufs=1, space="PSUM"))
    ztp = ctx.enter_context(tc.tile_pool(name="ztp", bufs=1, space="PSUM"))

    # zt bank: folds use cols 0..143; PE-warming fillers overwrite the whole
    # bank before any fold runs (start=True resets the entries they touch)
    zt = ztp.tile([128, 512], F32, tag="zt")

    def fill(n):
        """p-state warmers: keep the PE streaming during input-DMA / staging
        waits so the clock ramp (full speed needs ~3us of continuous busy)
        completes before the real work arrives. Depends only on local
        memsets; mathematically dead (folds re-reset their entries)."""
        for _ in range(n):
            nc.tensor.matmul(zt[:, :], zrow[:], frow[:],
                             start=True, stop=True, skip_group_check=True)

    fill(FILL_PRE)

    # ---- Q projection: fp8 DoubleRow, all 4 head-pairs in one stp tile ----
    pq = stp.tile([128, 1024], F32, tag="st")
    for d in range(4):
        nc.tensor.matmul(pq[:, d * NQ:(d + 1) * NQ],
                         wq8[:, :, d * 128:(d + 1) * 128],
                         xq8[:], start=True, stop=True, perf_mode=DR)
    fill(FILL_Q)
    for d in range(4):
        if d % 2 == 0:
            nc.scalar.activation(QT[d][:], pq[:, d * NQ:(d + 1) * NQ],
                                 AF.Identity,
                                 bias=bq_col[:, d:d + 1], scale=1.0)
        else:
            nc.vector.tensor_scalar_add(QT[d][:], pq[:, d * NQ:(d + 1) * NQ],
                                        bq_col[:, d:d + 1])

    def kproj_mm(d, h):
        """half h: key chunks i = 0,1 -> one [128,1024] stp tile (fp8 DR)"""
        pk = stp.tile([128, 1024], F32, tag="st")
        for i in range(2):
            nc.tensor.matmul(pk[:, i * 512:(i + 1) * 512],
                             wk8[:, :, d * 128:(d + 1) * 128],
                             xT8h[h][i][:],
                             start=True, stop=True, perf_mode=DR)
        return pk

    def kstage(d, h, pk):
        if d == 0:
            # d0 staging is on the critical path to the first scores --
            # split each half across both engines and into separate tiles
            nc.scalar.activation(KT[d][h][:, 0:512], pk[:, 0:512], AF.Copy,
                                 bias=0.0, scale=1.0)
            nc.vector.tensor_copy(KT[d][h][:, 512:1024], pk[:, 512:1024])
        elif KT_ON_ACT[h]:
            nc.scalar.activation(KT[d][h][:], pk[:], AF.Copy,
                                 bias=0.0, scale=1.0)
        else:
            nc.vector.tensor_copy(KT[d][h][:], pk[:])

    def kproj(d, h):
        kstage(d, h, kproj_mm(d, h))

    # K projection for d=0 (d+1 is projected during d)
    kproj(0, 0)
    kproj(0, 1)
    fill(FILL_K)

    def scores_tile(d, t):
        """two key tiles (kt=2t, 2t+1), both heads -> st [128, 1024].
        Layout [h0kt0|h0kt1|h1kt0|h1kt1]: the concurrently-running
        head-pair matmuls (row groups 0/64) land in different banks --
        concurrent PE writes into one bank are a device fault."""
        st = stp.tile([128, 1024], F32, tag="st")
        for j in range(2):
            kt = 2 * t + j
            h, o = kt // 8, (kt % 8) * 128
            for hh in range(2):
                nc.tensor.matmul(
                    st[:, hh * 512 + j * NQ:hh * 512 + (j + 1) * NQ],
                    KT[d][h][hh * HD:(hh + 1) * HD, o:o + 128],
                    QT[d][hh * HD:(hh + 1) * HD, :])
        return st

    def exp_tile(d, t, st):
        p = PP[d][t]
        if d == 3 and t >= 6:
            # tail: halve latency by splitting the last tiles across engines
            nc.scalar.activation(p[:, 0:512], st[:, 0:512], AF.Exp,
                                 bias=0.0, scale=SC_EXP)
            nc.vector.tensor_scalar(p[:, 512:1024].bitcast(I16),
                                    st[:, 512:1024],
                                    B_IMM, A_IMM, op0=OP.add, op1=OP.mult)
            return
        if EXP_ENG[t] == "act":
            nc.scalar.activation(p[:], st[:], AF.Exp, bias=0.0, scale=SC_EXP)
        else:
            nc.vector.tensor_scalar(p[:].bitcast(I16), st[:],
                                    B_IMM, A_IMM, op0=OP.add, op1=OP.mult)

    def zw_batch(d, zw_d, b):
        """key tiles 4b..4b+3 -> 4 col-tiled strip matmuls. Batch 0 uses a
        32-wide stationary with start=True: resets the full 32-row strip
        (rows 9..31 get finite garbage that the E-fold multiplies by 0)."""
        for s in range(4):
            kt = 4 * b + s
            p = PP[d][kt // 2]
            j = kt % 2
            # moving = [h0 block j | h1 block j]: [128, 2, 256] AP
            pv = p[:].rearrange("p (h jq) -> p h jq", h=2)
            pv = pv[:, :, j * NQ:(j + 1) * NQ]
            so = 32 * s
            if b == 0:
                nc.tensor.matmul(
                    zw_d[so:so + 32, :],
                    uw[:, kt * 9:kt * 9 + 32],
                    pv,
                    start=True, stop=False,
                    tile_position=(0, so), skip_group_check=True)
            else:
                nc.tensor.matmul(
                    zw_d[so:so + 9, :],
                    uw[:, kt * 9:kt * 9 + 9],
                    pv,
                    start=False, stop=(kt == NKT - 1),
                    tile_position=(0, so), skip_group_check=True)

    def zw_store(d, zw_d, on_act=True):
        if d == 3:
            # tail: split across engines to halve the last store's latency
            nc.scalar.activation(zw_sb[:, d * 512:d * 512 + 256],
                                 zw_d[:, 0:256], AF.Copy, bias=0.0, scale=1.0)
            nc.vector.tensor_copy(zw_sb[:, d * 512 + 256:(d + 1) * 512],
                                  zw_d[:, 256:512])
        else:
            nc.scalar.activation(zw_sb[:, d * 512:(d + 1) * 512], zw_d[:],
                                 AF.Copy, bias=0.0, scale=1.0)

    def folds(dd):
        for ch in range(4):
            nc.tensor.matmul(
                zt[:, (4 * dd + ch) * 9:(4 * dd + ch) * 9 + 9],
                zw_sb[:, dd * 512 + ch * 128:dd * 512 + (ch + 1) * 128],
                efold[:], skip_group_check=True)

    # ---- main pipeline ----
    # zw batches lag the score/exp stream by half a d-iteration: every
    # batch consumes exp tiles issued >=2us earlier, so the PE never
    # stalls on a freshly-minted exp. Tail does the last two batches.
    zw = {}

    def zw_b(d, b):
        if b == 0:
            zw[d] = zwp.tile([128, 512], F32, tag="zw", name=f"zw{d}")
        zw_batch(d, zw[d], b)

    for d in range(4):
        def tile_(t):
            exp_tile(d, t, scores_tile(d, t))
        tile_(0)
        tile_(1)
        if d:
            zw_b(d - 1, 2)
        tile_(2)
        if d < 3:
            kproj(d + 1, 0)
        tile_(3)
        if d:
            zw_b(d - 1, 3)
            zw_store(d - 1, zw[d - 1], True)
        tile_(4)
        tile_(5)
        zw_b(d, 0)
        if d:
            folds(d - 1)
        if d < 3:
            kproj(d + 1, 1)
        tile_(6)
        tile_(7)
        zw_b(d, 1)
    zw_b(3, 2)
    zw_b(3, 3)
    zw_store(3, zw[3])
    folds(3)

    # ---- final combine: out[q] = sum_h W[q,h]/Z[q,h] + bo ----
    # zt col = 18H + 9qc + r (H = head, qc = query half, r = 0:Z, 1+H:W)
    ld = ctx.enter_context(tc.tile_pool(name="ld", bufs=1))
    zr = ld.tile([128, 16], F32, tag="zr")
    nc.vector.reciprocal(zr[:], zt[:, 0:136:9])           # (H, qc) pairs
    w_ap = zt[:, 1:1 + 19 * 8].rearrange("p (h r) -> p h r", h=8)[:, :, 0:10:9]
    wz = ld.tile([128, 16], F32, tag="wz")                # layout [qc, H]
    nc.vector.tensor_mul(wz[:].rearrange("p (q h) -> p h q", q=2), w_ap,
                         zr[:].rearrange("p (h q) -> p h q", h=8))
    sm = ld.tile([128, 2], F32, tag="sm")
    nc.vector.reduce_sum(sm[:], wz[:].rearrange("p (q h) -> p q h", q=2),
                         axis=mybir.AxisListType.X)
    nc.vector.tensor_scalar_add(res[:], sm[:], bo_rep[:])
    nc.sync.dma_start(d_out.rearrange("(q p) o -> p (q o)", p=128), res[:])


def _host_prep(inputs):
    f32 = np.float32
    bf = ml_dtypes.bfloat16
    f8 = ml_dtypes.float8_e4m3
    x = np.ascontiguousarray(inputs["x"], dtype=f32)
    Wo0 = inputs["Wo"][:, 0].astype(f32)
    wv_t = (inputs["Wv"].astype(f32) * Wo0[None, :]).reshape(CIN, H, HD).sum(-1)
    bv_t = (inputs["bv"].astype(f32) * Wo0).reshape(H, HD).sum(-1)
    # motion gate (host: O(N*small) input prep)
    mf = np.concatenate([inputs["rel_vel"], inputs["rel_angle"]], 1).astype(f32)
    z = np.maximum(mf @ inputs["Wmg1"].astype(f32) + inputs["bmg1"], 0.0)
    z = z @ inputs["Wmg2"].astype(f32) + inputs["bmg2"]
    mg = 1.0 / (1.0 + np.exp(-z))                      # (N, 1)
    U = mg * (x @ wv_t + bv_t)                         # (N, 8) gated
    uw_full = np.concatenate([np.ones((N, 1), f32), U], 1)   # (N, 9)
    uw_pack = uw_full.reshape(NKT, 128, 9).transpose(1, 0, 2).reshape(128, -1)
    E = np.zeros((128, 9), f32)
    for s in range(4):
        E[32 * s:32 * s + 9, :] = np.eye(9, dtype=f32)
    xt = np.ascontiguousarray(x.T)                     # (256, 2048) f32
    wq = inputs["Wq"].astype(f32)
    wk = inputs["Wk"].astype(f32)
    pfv = np.concatenate(
        [inputs["bq"].astype(f32).reshape(4, 128).T,
         np.full((128, 1), inputs["bo"][0], f32)], axis=1)
    common = dict(
        xt_local=xt,
        pack_f32=np.ascontiguousarray(pfv),
    )
    for h in range(2):
        common[f"pack_x{h}"] = np.ascontiguousarray(
            np.concatenate([xt[0:128, h * 1024:(h + 1) * 1024],
                            xt[128:256, h * 1024:(h + 1) * 1024]],
                           axis=1)).astype(f8)
    common["pack_k8"] = np.ascontiguousarray(
        np.concatenate([wk[0:128], wk[128:256]], axis=1)).astype(f8)
    common["pack_k"] = np.ascontiguousarray(
        np.concatenate([uw_pack, E], axis=1)).astype(bf)
    common["_wq8"] = np.concatenate([wq[0:128], wq[128:256]], axis=1)
    return common


def kernel(**inputs):
    if "nc" not in _CACHE:
        _CACHE["nc"] = _build_nc()
    nc = _CACHE["nc"]
    common = _host_prep(inputs)
    xt = common.pop("xt_local")
    wq8 = common.pop("_wq8")
    f8 = ml_dtypes.float8_e4m3
    in_maps = []
    for i in range(NCORES):
        xq = xt[:, i * NQ:(i + 1) * NQ]
        xq8 = np.concatenate([xq[0:128], xq[128:256]], axis=1)
        packed = np.concatenate([xq8, wq8], axis=1)
        in_maps.append(dict(common,
                            pack_q8=np.ascontiguousarray(packed).astype(f8)))
    res = run_bass_kernel_spmd(nc, in_maps, core_ids=list(range(NCORES)),
                               **_CACHE.get("run_kwargs", {}))
    _CACHE["last_results"] = res
    out = np.concatenate([np.asarray(res.results[i]["out"])[:, 0]
                          for i in range(NCORES)])
    return out.astype(np.float32)
